# revision 6
# baseline (speedup 1.0000x reference)
"""EnhancedCAREGNN Trainium2 kernel (8 NeuronCores, node-sharded, no collectives).

Strategy
--------
* Nodes are packed into bins ("tiles") of <=128 nodes with <=640 edges per
  view (so every (tile, view) needs exactly NB=5 edge blocks of 128 edges).
  Bins are distributed over the 8 cores; each core processes only edges whose
  *destination* lands in its bins, so no cross-core reduction is needed.
* segment_sum is done on the tensor engine: for each 128-edge block we build
  a one-hot(dst)*w matrix with iota+tensor_scalar and accumulate
  G_block.T @ onehotw into a PSUM tile holding S.T = (sum_e w_e * F[src_e]).T
  for a 512-node supertile.  Linearity gives
  agg = S @ relW + relb * segsum(w), so the per-edge matmul of the reference
  collapses into one per-supertile matmul.
* All node-level MLPs / attentions / layernorm run in transposed layout
  [feature, node] so biases are per-partition and no on-chip transposes are
  needed.  Partition-dim reductions (layernorm mean/var, bias broadcast) are
  done with rank-1 / ones matmuls on the tensor engine.
* The host only reorganizes data (sorting/partitioning edges, permuting node
  ids, duplicating rows of `features` into per-edge-block order).  All
  arithmetic on values happens on the device.
"""

import os

import numpy as np

import concourse.bacc as bacc
import concourse.tile as tile
from concourse import mybir
from concourse.bass_utils import run_bass_kernel_spmd

# problem constants (from the problem spec; the harness always calls with
# these shapes)
N = 100000
D = 128
DO = 128
V = 3
C = 2
H = 64
VH = 64
NCORES = 8
P = 128
NB = 5            # edge blocks per (tile, view);  tile edge cap = NB*128
EPS = 1e-5

F32 = mybir.dt.float32
AF = mybir.ActivationFunctionType
OP = mybir.AluOpType

LAST_EXEC_NS = None


# --------------------------------------------------------------------------
# host-side data layout
# --------------------------------------------------------------------------

def _pack_bins(deg):
    """Assign nodes 0..N-1 sequentially to bins of <=128 nodes with
    per-view edge count <= NB*128.  Returns (bin_id, slot) per node."""
    cap = NB * P
    cum = np.cumsum(deg, axis=1)          # [V, N]
    czero = np.zeros((V, 1), dtype=cum.dtype)
    cum0 = np.concatenate([czero, cum], axis=1)   # cum0[v, i] = sum deg[:i]
    bin_id = np.empty(N, np.int32)
    slot = np.empty(N, np.int32)
    start = 0
    b = 0
    while start < N:
        take = min(P, N - start)
        # shrink until every view fits in cap
        while take > 1:
            ok = True
            for v in range(V):
                if cum0[v, start + take] - cum0[v, start] > cap:
                    ok = False
                    break
            if ok:
                break
            # binary-search-ish shrink: step down proportionally
            take -= max(1, take // 8)
        bin_id[start:start + take] = b
        slot[start:start + take] = np.arange(take)
        start += take
        b += 1
    return bin_id, slot, b


def _build_program(TILES, tis_list, need_relb, attb2_v, vattb2_v, att_bias_v):
    """Build the SPMD Bacc program.  All structural parameters are shared by
    the 8 cores."""
    NST = len(tis_list)
    TOTBLK = TILES * V * NB
    NCOL = TILES * P

    nc = bacc.Bacc("TRN2", target_bir_lowering=False, debug=False,
                   num_devices=NCORES)

    # ---- dram tensors -----------------------------------------------------
    din = {}

    def dt(name, shape, dtype=F32, kind="ExternalInput"):
        din[name] = nc.dram_tensor(name, list(shape), dtype, kind=kind)
        return din[name]

    dt("gtab", [P, TOTBLK * P])
    dt("dwt", [P, TOTBLK])
    dt("wt", [P, TOTBLK])
    dt("featT", [P, NCOL])
    dt("iota", [P, P])
    dt("clsW", [D, C]); dt("clsb", [1, C])
    dt("attW1", [D, C, H]); dt("attb1", [H, C]); dt("attW2", [H, C])
    dt("relW", [D, V, DO])
    if need_relb:
        dt("relb", [1, V, DO])
    dt("gateW", [DO, V, DO]); dt("gateb", [DO, V])
    dt("view_pref", [DO, V])
    dt("vattW1", [DO, VH]); dt("vattb1", [VH, 1]); dt("vattW2", [VH, 1])
    dt("selfW", [D, DO]); dt("selfb", [DO, 1])
    dt("featW", [D, DO]); dt("featb", [DO, 1])
    dt("fusW1", [DO, DO]); dt("fusW2", [DO, DO]); dt("fusb", [DO, 1])
    dt("ln_g", [DO, 1]); dt("ln_b", [DO, 1])
    dt("attb2", [1, C])
    out_t = nc.dram_tensor("out_t", [P, NCOL], F32, kind="ExternalOutput")
    cp_t = nc.dram_tensor("cp_t", [C, NCOL], F32, kind="ExternalOutput")

    GW = 4 * NB * P  # gather buffer free size (max blocks/st/view * 128)

    with tile.TileContext(nc) as tc:
        with (
            tc.tile_pool(name="sing", bufs=1) as sing,
            tc.tile_pool(name="ld", bufs=2) as ld,
            tc.tile_pool(name="work", bufs=2) as wk,
            tc.tile_pool(name="eqwp", bufs=4) as eqwp,
            tc.tile_pool(name="rows", bufs=6) as rows,
            tc.tile_pool(name="ps_st", bufs=2, space="PSUM") as ps_st,
            tc.tile_pool(name="ps_big", bufs=3, space="PSUM") as ps_big,
            tc.tile_pool(name="ps_row", bufs=3, space="PSUM") as ps_row,
        ):
            # ---- load constants / weights into SBUF -----------------------
            def load(name, shape):
                t = sing.tile(list(shape), F32, tag=f"w_{name}")
                nc.sync.dma_start(out=t[:], in_=din[name][:])
                return t

            iota_t = load("iota", [P, P])
            dw_t = load("dwt", [P, TOTBLK])
            w_t = load("wt", [P, TOTBLK])
            clsW_t = load("clsW", [D, C]); clsb_t = load("clsb", [1, C])
            attW1_t = load("attW1", [D, C, H])
            attb1_t = load("attb1", [H, C])
            attW2_t = load("attW2", [H, C])
            relW_t = load("relW", [D, V, DO])
            relb_t = load("relb", [1, V, DO]) if need_relb else None
            gateW_t = load("gateW", [DO, V, DO])
            gateb_t = load("gateb", [DO, V])
            vpref_t = load("view_pref", [DO, V])
            vattW1_t = load("vattW1", [DO, VH])
            vattb1_t = load("vattb1", [VH, 1])
            vattW2_t = load("vattW2", [VH, 1])
            selfW_t = load("selfW", [D, DO]); selfb_t = load("selfb", [DO, 1])
            featW_t = load("featW", [D, DO]); featb_t = load("featb", [DO, 1])
            fusW1_t = load("fusW1", [DO, DO]); fusW2_t = load("fusW2", [DO, DO])
            fusb_t = load("fusb", [DO, 1])
            lng_t = load("ln_g", [DO, 1]); lnb_t = load("ln_b", [DO, 1])
            attb2_t = load("attb2", [1, C])
            eps_t = sing.tile([1, 1], F32)
            nc.vector.memset(eps_t[:], EPS)

            ones128 = sing.tile([P, 1], F32)      # column of ones (LN sums)
            nc.vector.memset(ones128[:], 1.0)
            ones_row = sing.tile([1, P], F32)     # row of ones (bcast matmul)
            nc.vector.memset(ones_row[:], 1.0)
            zcol = sing.tile([1, P], F32)         # zeros lhsT (psum zero-init)
            nc.vector.memset(zcol[:], 0.0)
            zrow = sing.tile([1, 512], F32)
            nc.vector.memset(zrow[:], 0.0)

            blk0 = 0   # running global block index
            col0 = 0   # running node-column offset
            eqw_flip = 0
            for st in range(NST):
                TIS = tis_list[st]
                W = TIS * P
                ft = ld.tile([P, 512], F32, tag="ft")
                nc.sync.dma_start(out=ft[:, :W],
                                  in_=din["featT"][:, col0:col0 + W])

                vouts = []
                for v in range(V):
                    nb = TIS * NB
                    gb = ld.tile([P, GW], F32, tag="g")
                    nc.sync.dma_start(
                        out=gb[:, :nb * P],
                        in_=din["gtab"][:, blk0 * P:(blk0 + nb) * P])

                    st_ps = ps_st.tile([P, 512], F32, tag="st")
                    # zero-init so untouched columns read 0
                    nc.tensor.matmul(out=st_ps[:, :W], lhsT=zcol[:],
                                     rhs=zrow[:, :W], start=True, stop=False)
                    if need_relb:
                        ws_ps = ps_row.tile([H, 512], F32, tag="row")
                        nc.tensor.matmul(out=ws_ps[:1, :W], lhsT=zcol[:, :1],
                                         rhs=zrow[:, :W], start=True,
                                         stop=False)
                    for tau in range(TIS):
                        for b in range(NB):
                            k = tau * NB + b
                            B = blk0 + k
                            eqw = eqwp.tile([P, P], F32, tag="eqw")
                            eng = nc.vector if (eqw_flip % 2 == 0) else nc.gpsimd
                            eqw_flip += 1
                            eng.tensor_scalar(
                                out=eqw[:], in0=iota_t[:],
                                scalar1=dw_t[:, B:B + 1],
                                scalar2=w_t[:, B:B + 1],
                                op0=OP.is_equal, op1=OP.mult)
                            last = (k == nb - 1)
                            nc.tensor.matmul(
                                out=st_ps[:, tau * P:(tau + 1) * P],
                                lhsT=gb[:, k * P:(k + 1) * P],
                                rhs=eqw[:], start=False,
                                stop=(last and not need_relb))
                            if need_relb:
                                nc.tensor.matmul(
                                    out=ws_ps[:1, tau * P:(tau + 1) * P],
                                    lhsT=ones128[:], rhs=eqw[:],
                                    start=False, stop=last)
                    sts = wk.tile([P, 512], F32, tag="sts")
                    nc.vector.tensor_copy(out=sts[:, :W], in_=st_ps[:, :W])

                    agg_ps = ps_big.tile([P, 512], F32, tag="big")
                    nc.tensor.matmul(out=agg_ps[:, :W], lhsT=relW_t[:, v, :],
                                     rhs=sts[:, :W], start=True,
                                     stop=not need_relb)
                    if need_relb:
                        wsr = rows.tile([1, 512], F32, tag="r1")
                        nc.vector.tensor_copy(out=wsr[:, :W],
                                              in_=ws_ps[:1, :W])
                        nc.tensor.matmul(out=agg_ps[:, :W], lhsT=relb_t[:, v, :],
                                         rhs=wsr[:, :W], start=False,
                                         stop=True)
                    agg_s = wk.tile([P, 512], F32, tag="agg")
                    nc.vector.tensor_copy(out=agg_s[:, :W], in_=agg_ps[:, :W])

                    gate_ps = ps_big.tile([P, 512], F32, tag="big")
                    nc.tensor.matmul(out=gate_ps[:, :W], lhsT=gateW_t[:, v, :],
                                     rhs=agg_s[:, :W], start=True, stop=True)
                    gate_s = wk.tile([P, 512], F32, tag="gate")
                    nc.scalar.activation(out=gate_s[:, :W], in_=gate_ps[:, :W],
                                         func=AF.Sigmoid, bias=gateb_t[:, v:v + 1])
                    vout = wk.tile([P, 512], F32, tag=f"vout{v}")
                    nc.vector.tensor_mul(out=vout[:, :W], in0=gate_s[:, :W],
                                         in1=agg_s[:, :W])
                    vouts.append(vout)

                    pre = wk.tile([P, 512], F32, tag="pre")
                    nc.vector.tensor_scalar_mul(out=pre[:, :W],
                                                in0=vout[:, :W],
                                                scalar1=vpref_t[:, v:v + 1])
                    vh_ps = ps_row.tile([H, 512], F32, tag="row")
                    nc.tensor.matmul(out=vh_ps[:VH, :W], lhsT=vattW1_t[:],
                                     rhs=pre[:, :W], start=True, stop=True)
                    vh_s = wk.tile([VH, 512], F32, tag="vh")
                    nc.scalar.activation(out=vh_s[:, :W], in_=vh_ps[:VH, :W],
                                         func=AF.Relu, bias=vattb1_t[:])
                    vs_ps = ps_row.tile([H, 512], F32, tag="row")
                    nc.tensor.matmul(out=vs_ps[:1, :W], lhsT=vattW2_t[:],
                                     rhs=vh_s[:, :W], start=True, stop=True)
                    if v == 0:
                        vsc = []
                    vsc_v = wk.tile([1, 512], F32, tag=f"vsc{v}")
                    vsc.append(vsc_v)
                    nc.vector.tensor_copy(out=vsc_v[:, :W],
                                          in_=vs_ps[:1, :W])
                    blk0 += nb

                # ---- label-aware attention (per-class [1, W] rows) ---------
                lgr = []
                for cc in range(C):
                    lg_ps = ps_row.tile([H, 512], F32, tag="row")
                    nc.tensor.matmul(out=lg_ps[:1, :W],
                                     lhsT=clsW_t[:, cc:cc + 1],
                                     rhs=ft[:, :W], start=True, stop=True)
                    lg_c = wk.tile([1, 512], F32, tag=f"lg{cc}")
                    nc.vector.tensor_scalar_add(out=lg_c[:, :W],
                                                in0=lg_ps[:1, :W],
                                                scalar1=clsb_t[:, cc:cc + 1])
                    lgr.append(lg_c)
                pm = rows.tile([1, 512], F32, tag="r1")
                nc.vector.tensor_tensor(out=pm[:, :W], in0=lgr[0][:, :W],
                                        in1=lgr[1][:, :W], op=OP.max)
                probs = []
                for cc in range(C):
                    tr = rows.tile([1, 512], F32, tag="r1")
                    nc.vector.tensor_sub(out=tr[:, :W], in0=lgr[cc][:, :W],
                                         in1=pm[:, :W])
                    pr_c = wk.tile([1, 512], F32, tag=f"probs{cc}")
                    nc.scalar.activation(out=pr_c[:, :W],
                                         in_=tr[:, :W], func=AF.Exp)
                    probs.append(pr_c)
                psum_r = rows.tile([1, 512], F32, tag="r1")
                nc.vector.tensor_add(out=psum_r[:, :W], in0=probs[0][:, :W],
                                     in1=probs[1][:, :W])
                prec = rows.tile([1, 512], F32, tag="r1")
                nc.vector.reciprocal(out=prec[:, :W], in_=psum_r[:, :W])
                for cc in range(C):
                    nc.vector.tensor_mul(out=probs[cc][:, :W],
                                         in0=probs[cc][:, :W],
                                         in1=prec[:, :W])
                    nc.sync.dma_start(out=cp_t[cc:cc + 1, col0:col0 + W],
                                      in_=probs[cc][:, :W])

                scs = []
                for cc in range(C):
                    h_ps = ps_row.tile([H, 512], F32, tag="row")
                    nc.tensor.matmul(out=h_ps[:H, :W], lhsT=attW1_t[:, cc, :],
                                     rhs=ft[:, :W], start=True, stop=True)
                    h_s = wk.tile([H, 512], F32, tag="hs")
                    nc.scalar.activation(out=h_s[:, :W], in_=h_ps[:H, :W],
                                         func=AF.Relu, bias=attb1_t[:, cc:cc + 1])
                    sc_ps = ps_row.tile([H, 512], F32, tag="row")
                    nc.tensor.matmul(out=sc_ps[:1, :W], lhsT=attW2_t[:, cc:cc + 1],
                                     rhs=h_s[:, :W], start=True, stop=True)
                    sc_c = wk.tile([1, 512], F32, tag=f"scs{cc}")
                    nc.scalar.activation(out=sc_c[:, :W],
                                         in_=sc_ps[:1, :W], func=AF.Sigmoid,
                                         bias=attb2_t[:, cc:cc + 1])
                    scs.append(sc_c)
                ar = rows.tile([1, 512], F32, tag="r1")
                t1r = rows.tile([1, 512], F32, tag="r1")
                nc.vector.tensor_mul(out=ar[:, :W], in0=probs[0][:, :W],
                                     in1=scs[0][:, :W])
                nc.vector.tensor_mul(out=t1r[:, :W], in0=probs[1][:, :W],
                                     in1=scs[1][:, :W])
                nc.vector.tensor_add(out=ar[:, :W], in0=ar[:, :W],
                                     in1=t1r[:, :W])
                nc.vector.tensor_scalar_add(out=ar[:, :W], in0=ar[:, :W],
                                            scalar1=float(att_bias_v))

                # ---- view softmax + attention weighting -------------------
                # (vattb2 shifts all view scores equally; softmax over views
                # is invariant to it, so it is omitted.)
                m01 = rows.tile([1, 512], F32, tag="r1")
                nc.vector.tensor_tensor(out=m01[:, :W], in0=vsc[0][:, :W],
                                        in1=vsc[1][:, :W], op=OP.max)
                nc.vector.tensor_tensor(out=m01[:, :W], in0=m01[:, :W],
                                        in1=vsc[2][:, :W], op=OP.max)
                uv = []
                for v in range(V):
                    uv_v = wk.tile([1, 512], F32, tag=f"uv{v}")
                    uv.append(uv_v)
                for v in range(V):
                    tr = rows.tile([1, 512], F32, tag="r1")
                    nc.vector.tensor_sub(out=tr[:, :W], in0=vsc[v][:, :W],
                                         in1=m01[:, :W])
                    nc.scalar.activation(out=uv[v][:, :W], in_=tr[:, :W],
                                         func=AF.Exp)
                vsum = rows.tile([1, 512], F32, tag="r1")
                nc.vector.tensor_add(out=vsum[:, :W], in0=uv[0][:, :W],
                                     in1=uv[1][:, :W])
                nc.vector.tensor_add(out=vsum[:, :W], in0=vsum[:, :W],
                                     in1=uv[2][:, :W])
                vrec = rows.tile([1, 512], F32, tag="r1")
                nc.vector.reciprocal(out=vrec[:, :W], in_=vsum[:, :W])
                # fold node_att into the per-view weights: u_v = e_v/sum * att
                nc.vector.tensor_mul(out=vrec[:, :W], in0=vrec[:, :W],
                                     in1=ar[:, :W])
                for v in range(V):
                    nc.vector.tensor_mul(out=uv[v][:, :W],
                                         in0=uv[v][:, :W],
                                         in1=vrec[:, :W])

                comb = wk.tile([P, 512], F32, tag="comb")
                for v in range(V):
                    ub_ps = ps_big.tile([P, 512], F32, tag="big")
                    nc.tensor.matmul(out=ub_ps[:, :W], lhsT=ones_row[:],
                                     rhs=uv[v][:, :W], start=True,
                                     stop=True)
                    if v == 0:
                        nc.vector.tensor_mul(out=comb[:, :W],
                                             in0=vouts[0][:, :W],
                                             in1=ub_ps[:, :W])
                    else:
                        cm = wk.tile([P, 512], F32, tag="cmix")
                        nc.vector.tensor_mul(out=cm[:, :W],
                                             in0=vouts[v][:, :W],
                                             in1=ub_ps[:, :W])
                        nc.vector.tensor_add(out=comb[:, :W],
                                             in0=comb[:, :W], in1=cm[:, :W])

                # ---- fusion + residual + layernorm -------------------------
                self_ps = ps_big.tile([P, 512], F32, tag="big")
                nc.tensor.matmul(out=self_ps[:, :W], lhsT=selfW_t[:],
                                 rhs=ft[:, :W], start=True, stop=True)
                self_s = wk.tile([P, 512], F32, tag="selfs")
                nc.vector.tensor_scalar_add(out=self_s[:, :W],
                                            in0=self_ps[:, :W],
                                            scalar1=selfb_t[:])
                fus_ps = ps_big.tile([P, 512], F32, tag="big")
                nc.tensor.matmul(out=fus_ps[:, :W], lhsT=fusW1_t[:],
                                 rhs=self_s[:, :W], start=True, stop=False)
                nc.tensor.matmul(out=fus_ps[:, :W], lhsT=fusW2_t[:],
                                 rhs=comb[:, :W], start=False, stop=True)
                fused_s = wk.tile([P, 512], F32, tag="fused")
                nc.scalar.activation(out=fused_s[:, :W], in_=fus_ps[:, :W],
                                     func=AF.Relu, bias=fusb_t[:])
                tr_ps = ps_big.tile([P, 512], F32, tag="big")
                nc.tensor.matmul(out=tr_ps[:, :W], lhsT=featW_t[:],
                                 rhs=ft[:, :W], start=True, stop=True)
                outp = wk.tile([P, 512], F32, tag="outp")
                nc.vector.tensor_scalar_add(out=outp[:, :W],
                                            in0=tr_ps[:, :W],
                                            scalar1=featb_t[:])
                nc.vector.tensor_add(out=outp[:, :W], in0=outp[:, :W],
                                     in1=fused_s[:, :W])

                # layernorm over the feature (partition) dim via ones-matmul
                sq = wk.tile([P, 512], F32, tag="sq")
                nc.vector.tensor_mul(out=sq[:, :W], in0=outp[:, :W],
                                     in1=outp[:, :W])
                sum_ps = ps_row.tile([H, 512], F32, tag="row")
                nc.tensor.matmul(out=sum_ps[:1, :W], lhsT=ones128[:],
                                 rhs=outp[:, :W], start=True, stop=True)
                ssq_ps = ps_row.tile([H, 512], F32, tag="row")
                nc.tensor.matmul(out=ssq_ps[:1, :W], lhsT=ones128[:],
                                 rhs=sq[:, :W], start=True, stop=True)
                mu = rows.tile([1, 512], F32, tag="r1")
                nc.vector.tensor_scalar_mul(out=mu[:, :W],
                                            in0=sum_ps[:1, :W],
                                            scalar1=1.0 / DO)
                var = rows.tile([1, 512], F32, tag="r1")
                nc.vector.tensor_scalar_mul(out=var[:, :W],
                                            in0=ssq_ps[:1, :W],
                                            scalar1=1.0 / DO)
                mu2 = rows.tile([1, 512], F32, tag="r1")
                nc.vector.tensor_mul(out=mu2[:, :W], in0=mu[:, :W],
                                     in1=mu[:, :W])
                nc.vector.tensor_sub(out=var[:, :W], in0=var[:, :W],
                                     in1=mu2[:, :W])
                std = rows.tile([1, 512], F32, tag="r1")
                nc.scalar.activation(out=std[:, :W], in_=var[:, :W],
                                     func=AF.Sqrt, bias=eps_t[:])
                rstd = rows.tile([1, 512], F32, tag="r1")
                nc.vector.reciprocal(out=rstd[:, :W], in_=std[:, :W])
                brow = rows.tile([1, 512], F32, tag="r1")
                nc.vector.tensor_mul(out=brow[:, :W], in0=mu[:, :W],
                                     in1=rstd[:, :W])
                a_ps = ps_big.tile([P, 512], F32, tag="big")
                nc.tensor.matmul(out=a_ps[:, :W], lhsT=ones_row[:],
                                 rhs=rstd[:, :W], start=True, stop=True)
                b_ps = ps_big.tile([P, 512], F32, tag="big")
                nc.tensor.matmul(out=b_ps[:, :W], lhsT=ones_row[:],
                                 rhs=brow[:, :W], start=True, stop=True)
                fin = wk.tile([P, 512], F32, tag="fin")
                nc.vector.tensor_mul(out=fin[:, :W], in0=outp[:, :W],
                                     in1=a_ps[:, :W])
                nc.vector.tensor_sub(out=fin[:, :W], in0=fin[:, :W],
                                     in1=b_ps[:, :W])
                nc.vector.tensor_scalar(out=fin[:, :W], in0=fin[:, :W],
                                        scalar1=lng_t[:], scalar2=lnb_t[:],
                                        op0=OP.mult, op1=OP.add)
                nc.sync.dma_start(out=out_t[:, col0:col0 + W],
                                  in_=fin[:, :W])
                col0 += W

    nc.compile()
    return nc


def kernel(**inputs):
    global LAST_EXEC_NS
    inp = {k: np.asarray(v) for k, v in inputs.items()}
    features = inp["features"].astype(np.float32)
    edge_src = inp["edge_src"].astype(np.int64)
    edge_dst = inp["edge_dst"].astype(np.int64)
    edge_w = inp["edge_w"].astype(np.float32)

    # ---- node -> (core, tile, slot) packing -------------------------------
    deg = np.stack([np.bincount(edge_dst[v], minlength=N) for v in range(V)])
    bin_id, slot, nbins = _pack_bins(deg)
    TILES = -(-nbins // NCORES)
    core_of_bin = np.arange(NCORES * TILES) // TILES
    tile_of_bin = np.arange(NCORES * TILES) % TILES
    node_core = core_of_bin[bin_id]
    node_tile = tile_of_bin[bin_id]
    node_col = node_tile * P + slot

    NST = -(-TILES // 4)
    tis_list = [4] * (NST - 1) + [TILES - 4 * (NST - 1)]
    TOTBLK = TILES * V * NB
    NCOL = TILES * P

    # block index layout must match the device loop: st -> v -> tile -> b
    blk_of_tile_view = np.empty((TILES, V), np.int64)
    blk0 = 0
    for st in range(NST):
        tis = tis_list[st]
        for v in range(V):
            for tau in range(tis):
                blk_of_tile_view[st * 4 + tau, v] = blk0 + tau * NB
            blk0 += tis * NB
    assert blk0 == TOTBLK

    # ---- per-core data ----------------------------------------------------
    relb = inp["relb"].astype(np.float32)
    need_relb = bool(np.abs(relb).max() > 0)

    feat_aug = np.concatenate(
        [features, np.zeros((1, D), np.float32)], axis=0)

    gtabs, dwts, wts, featTs = [], [], [], []
    for c in range(NCORES):
        gsrc = np.full((P, TOTBLK), N, np.int64)     # N -> zero row
        dwt = np.zeros((P, TOTBLK), np.float32)
        wt = np.zeros((P, TOTBLK), np.float32)
        for v in range(V):
            d = edge_dst[v]
            m = node_core[d] == c
            s_e = edge_src[v][m]
            w_e = edge_w[v][m]
            t_e = node_tile[d[m]]
            sl_e = (node_col[d[m]] % P).astype(np.int64)
            order = np.argsort(t_e, kind="stable")
            s_e, w_e, t_e, sl_e = s_e[order], w_e[order], t_e[order], sl_e[order]
            grp_start = np.searchsorted(t_e, np.arange(TILES))
            rank = np.arange(len(t_e)) - grp_start[t_e]
            assert rank.max(initial=0) < NB * P
            b_e = rank // P
            p_e = rank % P
            B_e = blk_of_tile_view[t_e, v] + b_e
            gsrc[p_e, B_e] = s_e
            dwt[p_e, B_e] = sl_e.astype(np.float32)
            wt[p_e, B_e] = w_e
        gtab = feat_aug[gsrc]                  # [P, TOTBLK, D]
        gtabs.append(np.ascontiguousarray(
            gtab.reshape(P, TOTBLK * D)))
        dwts.append(dwt)
        wts.append(wt)
        ftc = np.zeros((P, NCOL), np.float32)
        m = node_core == c
        ftc[:, node_col[m]] = features[m].T
        featTs.append(ftc)

    # ---- weights ----------------------------------------------------------
    iota = np.tile(np.arange(P, dtype=np.float32), (P, 1))
    fusW = inp["fusW"].astype(np.float32)
    wmap = {
        "iota": iota,
        "clsW": inp["clsW"].astype(np.float32),
        "clsb": inp["clsb"].astype(np.float32).reshape(1, C),
        "attW1": np.ascontiguousarray(
            np.transpose(inp["attW1"].astype(np.float32), (1, 0, 2))),
        "attb1": np.ascontiguousarray(inp["attb1"].astype(np.float32).T),
        "attW2": np.ascontiguousarray(inp["attW2"].astype(np.float32).T),
        "relW": np.ascontiguousarray(
            np.transpose(inp["relW"].astype(np.float32), (1, 0, 2))),
        "gateW": np.ascontiguousarray(
            np.transpose(inp["gateW"].astype(np.float32), (1, 0, 2))),
        "gateb": np.ascontiguousarray(inp["gateb"].astype(np.float32).T),
        "view_pref": np.ascontiguousarray(
            inp["view_pref"].astype(np.float32).T),
        "vattW1": inp["vattW1"].astype(np.float32),
        "vattb1": inp["vattb1"].astype(np.float32).reshape(VH, 1),
        "vattW2": inp["vattW2"].astype(np.float32).reshape(VH, 1),
        "selfW": inp["selfW"].astype(np.float32),
        "selfb": inp["selfb"].astype(np.float32).reshape(DO, 1),
        "featW": inp["featW"].astype(np.float32),
        "featb": inp["featb"].astype(np.float32).reshape(DO, 1),
        "fusW1": np.ascontiguousarray(fusW[:DO]),
        "fusW2": np.ascontiguousarray(fusW[DO:]),
        "fusb": inp["fusb"].astype(np.float32).reshape(DO, 1),
        "ln_g": inp["ln_g"].astype(np.float32).reshape(DO, 1),
        "ln_b": inp["ln_b"].astype(np.float32).reshape(DO, 1),
        "attb2": inp["attb2"].astype(np.float32).reshape(1, C),
    }
    if need_relb:
        wmap["relb"] = np.ascontiguousarray(relb.reshape(1, V, DO))

    nc = _build_program(
        TILES, tis_list, need_relb,
        attb2_v=inp["attb2"].astype(np.float32),
        vattb2_v=float(inp["vattb2"]),
        att_bias_v=float(inp["att_bias"]))

    in_maps = []
    for c in range(NCORES):
        m = dict(wmap)
        m["gtab"] = gtabs[c]
        m["dwt"] = dwts[c]
        m["wt"] = wts[c]
        m["featT"] = featTs[c]
        in_maps.append(m)

    trace = bool(os.environ.get("KERNEL_TRACE"))
    res = run_bass_kernel_spmd(nc, in_maps, list(range(NCORES)),
                               trace=trace, trace_cores=[0] if trace else None)
    LAST_EXEC_NS = res.exec_time_ns

    out = np.empty((N, DO), np.float32)
    cp = np.empty((N, C), np.float32)
    for c in range(NCORES):
        m = node_core == c
        out[m] = res.results[c]["out_t"][:, node_col[m]].T
        cp[m] = res.results[c]["cp_t"][:, node_col[m]].T
    return out, cp
